# revision 15
# baseline (speedup 1.0000x reference)
"""Trainium2 Bass kernel for GaussMonom: out[n] = const * exp(-(x[n]-mean) @ cov @ (x[n]-mean)).

Strategy (memory-bound, trivially data-parallel):
  - Shard the N=16.7M points across 8 cores (2,097,152 points/core).
  - Per core, view the [per, 2] slab as [128, 32768] f32 (row-major), so each
    partition row holds 16384 points with (x0, x1) interleaved. Loads are fully
    contiguous per partition; x0/x1 are read on-chip via stride-2 APs.
  - Host-side, expand zeta to a polynomial in (x0, x1) and complete squares:
        zeta = a*(x0+p0)^2 + c*(x1+q0)^2 + b*x0*x1 + g2
    so the ScalarE (ACT) Square op absorbs the linear terms, and the final Exp
    absorbs the scale by -a, the constant g2, and ln(const). Per tile:
        3 ACT passes (Square, Square, Exp) + 3 DVE passes (STT, STT, TT-add),
    all overlapped with ~3 MiB/tile of DMA, which is the bottleneck.
"""

import math

import numpy as np

try:
    from concourse import bacc, bass, mybir, tile
    from concourse import bass_utils
except ImportError:  # path fallback for bare containers
    import sys

    sys.path.insert(0, "/opt/trn_rl_repo")
    from concourse import bacc, bass, mybir, tile
    from concourse import bass_utils

N_CORES = 8
P = 128  # SBUF partitions

# Toggled by test.py for profiling; harness uses the defaults.
TRACE = False
TRACE_KWARGS = {}
LAST_RESULTS = None

FP32 = mybir.dt.float32
MULT = mybir.AluOpType.mult
ADD = mybir.AluOpType.add
SQUARE = mybir.ActivationFunctionType.Square
EXP = mybir.ActivationFunctionType.Exp


def _tile_plan(W, CW):
    """Column offsets/widths: uniform CW tiles, with the last CW-wide chunk
    tapered (2048,1024,512,512) so the tail's compute+store latency shrinks."""
    taper = [CW // 2, CW // 4, CW // 8, CW // 8]
    plan = []
    off = 0
    for _ in range(W // CW - 1):
        plan.append((off, CW))
        off += CW
    for s in taper:
        plan.append((off, s))
        off += s
    assert off == W
    return plan


def _emit_fast(nc, x, y, W, CW, co):
    """zeta = a*(x0+p0)^2 + c*(x1+q0)^2 + b*x0*x1 + g2
    Z = A1 + (c/a)*A2 + (b/a)*x0*x1;  out = exp(-a*Z + (-g2 + ln K)).
    Requires a != 0, c != 0, K > 0.

    Engine budget per full tile (F=2048 pts/partition): ACT 3 passes
    (Square, Square, Exp ~5.7us), DVE 2 STT passes (q, z ~4.4us), Pool 1
    TensorTensor (x0*x1 — TensorScalarPtr is NOT legal on Pool in the v3
    ISA), vs ~8.7us of DMA — memory-bound. Loads issue on sync's HWDGE
    queue, stores on scalar's, so store issue never queues behind loads."""
    with tile.TileContext(nc) as tc:
        with (
            tc.tile_pool(name="cst", bufs=1) as cst_pool,
            tc.tile_pool(name="xin", bufs=4) as xin_pool,
            tc.tile_pool(name="tmpa", bufs=2) as tmpa_pool,
            tc.tile_pool(name="tmp", bufs=2) as tmp_pool,
            tc.tile_pool(name="oot", bufs=6) as out_pool,
        ):
            cb_p0 = cst_pool.tile([P, 1], FP32, tag="cb_p0")
            nc.gpsimd.memset(cb_p0[:], co["p0"])
            cb_q0 = cst_pool.tile([P, 1], FP32, tag="cb_q0")
            nc.gpsimd.memset(cb_q0[:], co["q0"])
            cb_e = cst_pool.tile([P, 1], FP32, tag="cb_e")
            nc.gpsimd.memset(cb_e[:], co["bias_e"])

            for off, cw in _tile_plan(W, CW):
                F = cw // 2
                xt = xin_pool.tile([P, cw], FP32, tag="xt")
                nc.sync.dma_start(xt[:], x[:, off : off + cw])
                x0 = xt[:, 0::2]
                x1 = xt[:, 1::2]

                # a1 first: it gates q, the longest downstream chain.
                a1 = tmp_pool.tile([P, F], FP32, tag="a1")
                nc.scalar.activation(a1[:], x0, SQUARE, bias=cb_p0[:], scale=1.0)
                a2 = tmpa_pool.tile([P, F], FP32, tag="a2")
                nc.scalar.activation(a2[:], x1, SQUARE, bias=cb_q0[:], scale=1.0)

                p3 = tmpa_pool.tile([P, F], FP32, tag="p3")
                nc.gpsimd.tensor_tensor(p3[:], x0, x1, MULT)
                q = tmp_pool.tile([P, F], FP32, tag="q")
                nc.vector.scalar_tensor_tensor(q[:], p3[:], co["b_a"], a1[:], MULT, ADD)
                z = tmp_pool.tile([P, F], FP32, tag="z")
                nc.vector.scalar_tensor_tensor(z[:], a2[:], co["c_a"], q[:], MULT, ADD)

                o = out_pool.tile([P, F], FP32, tag="o")
                nc.scalar.activation(o[:], z[:], EXP, bias=cb_e[:], scale=co["neg_a"])
                nc.scalar.dma_start(y[:, off // 2 : off // 2 + F], o[:])


def _emit_general(nc, x, y, W, CW, co):
    """Fallback for degenerate coefficients: direct evaluation, more passes."""
    F = CW // 2
    ntiles = W // CW
    with tile.TileContext(nc) as tc:
        with (
            tc.tile_pool(name="xin", bufs=3) as xin_pool,
            tc.tile_pool(name="tmp", bufs=2) as tmp_pool,
            tc.tile_pool(name="oot", bufs=3) as out_pool,
        ):
            for i in range(ntiles):
                xt = xin_pool.tile([P, CW], FP32)
                nc.sync.dma_start(xt[:], x[:, i * CW : (i + 1) * CW])
                x0 = xt[:, 0::2]
                x1 = xt[:, 1::2]

                d0 = tmp_pool.tile([P, F], FP32)
                nc.vector.tensor_scalar_add(d0[:], x0, -co["m0"])
                d1 = tmp_pool.tile([P, F], FP32)
                nc.vector.tensor_scalar_add(d1[:], x1, -co["m1"])
                s1 = tmp_pool.tile([P, F], FP32)
                nc.scalar.mul(s1[:], d0[:], co["a"])
                s2 = tmp_pool.tile([P, F], FP32)
                nc.vector.scalar_tensor_tensor(s2[:], d1[:], co["b"], s1[:], MULT, ADD)
                s3 = tmp_pool.tile([P, F], FP32)
                nc.vector.tensor_mul(s3[:], s2[:], d0[:])
                s4 = tmp_pool.tile([P, F], FP32)
                nc.vector.scalar_tensor_tensor(s4[:], d1[:], co["c"], d1[:], MULT, MULT)
                s5 = tmp_pool.tile([P, F], FP32)
                nc.vector.tensor_add(s5[:], s3[:], s4[:])
                e = tmp_pool.tile([P, F], FP32)
                nc.scalar.activation(e[:], s5[:], EXP, bias=0.0, scale=-1.0)
                o = out_pool.tile([P, F], FP32)
                nc.vector.tensor_scalar_mul(o[:], e[:], co["K"])
                nc.sync.dma_start(y[:, i * F : (i + 1) * F], o[:])


def _coefficients(mean, cov, const):
    m0, m1 = float(mean[0]), float(mean[1])
    a = float(cov[0, 0])
    b = float(cov[0, 1]) + float(cov[1, 0])
    c = float(cov[1, 1])
    K = float(const[0])
    # zeta = a x0^2 + b x0 x1 + c x1^2 + e x0 + f x1 + g
    e = -(2.0 * a * m0 + b * m1)
    f = -(b * m0 + 2.0 * c * m1)
    g = a * m0 * m0 + b * m0 * m1 + c * m1 * m1

    fast = abs(a) > 1e-30 and abs(c) > 1e-30 and K > 0.0
    co = {"m0": m0, "m1": m1, "a": a, "b": b, "c": c, "K": K}
    if fast:
        p0 = e / (2.0 * a)
        q0 = f / (2.0 * c)
        g2 = g - a * p0 * p0 - c * q0 * q0
        co.update(
            p0=p0,
            q0=q0,
            b_a=b / a,
            c_a=c / a,
            neg_a=-a,
            bias_e=-g2 + math.log(K),
        )
    return fast, co


_NC_CACHE = {}


def _build_cached(W, CW, fast, co):
    key = (W, CW, fast) + tuple(sorted(co.items()))
    nc = _NC_CACHE.get(key)
    if nc is None:
        nc = _build(W, CW, fast, co)
        _NC_CACHE[key] = nc
    return nc


def _build(W, CW, fast, co):
    nc = bacc.Bacc(
        "TRN2",
        target_bir_lowering=False,
        debug=False,
        enable_asserts=False,
        num_devices=N_CORES,
    )
    x = nc.dram_tensor("x", [P, W], FP32, kind="ExternalInput").ap()
    y = nc.dram_tensor("y", [P, W // 2], FP32, kind="ExternalOutput").ap()
    if fast:
        _emit_fast(nc, x, y, W, CW, co)
    else:
        _emit_general(nc, x, y, W, CW, co)
    nc.compile()
    return nc


def kernel(tensor, mean, cov, const):
    global LAST_RESULTS
    tensor = np.ascontiguousarray(tensor, dtype=np.float32)
    mean = np.asarray(mean, dtype=np.float32)
    cov = np.asarray(cov, dtype=np.float32)
    const = np.asarray(const, dtype=np.float32)

    n = tensor.shape[0]
    per = n // N_CORES
    W = per * 2 // P  # f32 elements per partition row, per core
    CW = 4096  # input columns per tile (2 MiB loads)
    assert n % N_CORES == 0 and (per * 2) % P == 0 and W % CW == 0, (
        "unsupported shape for hardcoded sharding"
    )

    fast, co = _coefficients(mean, cov, const)
    nc = _build_cached(W, CW, fast, co)

    in_maps = [
        {"x": tensor[i * per : (i + 1) * per].reshape(P, W)} for i in range(N_CORES)
    ]
    try:
        res = bass_utils.run_bass_kernel_spmd(
            nc,
            in_maps,
            core_ids=list(range(N_CORES)),
            trace=TRACE,
            **TRACE_KWARGS,
        )
    except ModuleNotFoundError:
        # NTFF profiling hook (antenv.axon_hooks) absent in this container;
        # rerun without tracing.
        res = bass_utils.run_bass_kernel_spmd(
            nc, in_maps, core_ids=list(range(N_CORES)), trace=False
        )
    LAST_RESULTS = res
    out = np.concatenate(
        [res.results[i]["y"].reshape(-1) for i in range(N_CORES)]
    ).astype(np.float32, copy=False)
    return out
